# revision 17
# baseline (speedup 1.0000x reference)
"""Trainium2 Bass kernel for the PolymorphicSNN problem.

Contract: kernel(**inputs) takes FULL (unsharded) numpy inputs and returns the
full output tuple (combined, reg_mem, poly_mem_new, mode_probs), matching the
single-device jax reference.

Sharding (8 NeuronCores, SPMD, no collectives):
  - W_in is row-sharded: core i computes x_proc[i*1024:(i+1)*1024] = W_in[rows] @ x.
    The matvec runs on the Vector engine via the fused tensor_tensor_reduce
    (multiply + per-partition reduce in one pass, fp32-exact) with W in its
    NATURAL layout (rows on partitions) and x broadcast across partitions --
    this streams W at DMA line rate without paying the tensor engine's slow
    fp32 weight loads.
  - The regular-Leaky elementwise + spike threshold runs on-device per shard.
  - conn is column-sharded with the SAME index range: core i computes the
    partial neuron_in = conn[:, rows] @ reg_spk[rows] on the tensor engine
    with reg_spk (exactly 0/1, bf16-exact) as the 1-column stationary operand
    and conn^T split into bf16 hi+lo moving operands (exact to 2^-18).
    The 8 partial vectors (8KB each) are summed on the host during
    unsharding -- this replaces the all-reduce.
  - The tiny per-neuron MLP/softmax/poly-Leaky tail (~100K flops vs 80 MFLOP
    for the matvecs) runs on the host in fp32.
"""

import numpy as np

N = 8192
P = 2048
N_CORES = 8
RPC = N // N_CORES        # rows per core (1024)
MC = RPC // 128           # row chunks per core (8)
FW = 2048                 # free-dim span per DVE reduce op
WPIECE = 4096             # W columns per DMA (2MB pieces)
NWP = N // WPIECE         # W DMA pieces per row chunk (2)
CC = RPC // 128           # conn contraction chunks per core (8)
NFR = P // 512            # conn psum column ranges (4)

SLOPE = 25.0
BETAS = np.array([0.3, 0.7, 0.5], dtype=np.float32)
THRS = np.array([1.0, 0.8, 1.2], dtype=np.float32)

_COMPILED = {}


def _build_nc():
    import concourse.mybir as mybir
    import concourse.tile as tile
    from concourse import bacc

    f32 = mybir.dt.float32
    bf16 = mybir.dt.bfloat16
    gt = mybir.AluOpType.is_gt
    mult = mybir.AluOpType.mult
    add = mybir.AluOpType.add

    nc = bacc.Bacc("TRN2", target_bir_lowering=False, debug=False,
                   num_devices=N_CORES)

    x_f = nc.declare_dram_parameter("x_f", [1, N], f32, isOutput=False)
    mem_t = nc.declare_dram_parameter("mem_t", [128, MC], f32, isOutput=False)
    b_t = nc.declare_dram_parameter("b_t", [128, MC], f32, isOutput=False)
    wt = nc.declare_dram_parameter("wt", [MC, 128, N], f32, isOutput=False)
    ct_hi = nc.declare_dram_parameter("ct_hi", [CC, 128, P], bf16, isOutput=False)
    ct_lo = nc.declare_dram_parameter("ct_lo", [CC, 128, P], bf16, isOutput=False)
    o_rm = nc.declare_dram_parameter("o_rm", [128, MC], f32, isOutput=True)
    o_spk = nc.declare_dram_parameter("o_spk", [128, MC], f32, isOutput=True)
    o_nin = nc.declare_dram_parameter("o_nin", [1, P], f32, isOutput=True)

    with tile.TileContext(nc) as tc:
        with tc.tile_pool(name="small", bufs=1) as sp, \
             tc.tile_pool(name="w", bufs=3) as wp, \
             tc.tile_pool(name="c", bufs=CC) as cp, \
             tc.tile_pool(name="scratch", bufs=2) as scp, \
             tc.tile_pool(name="ps", bufs=4, space="PSUM") as pp:

            # x broadcast across all 128 partitions. Staging DMAs go on the
            # Scalar engine's HWDGE queue so the Sync queue (which carries the
            # bulk W/conn stream) is never blocked behind the broadcasts.
            xb = sp.tile([128, N], f32)
            XPC = N // 4
            for xi in range(4):
                xf = scp.tile([1, XPC], f32, tag="xf", name=f"xf{xi}", bufs=2)
                nc.scalar.dma_start(out=xf[:], in_=x_f[:, xi * XPC:(xi + 1) * XPC])
                nc.gpsimd.partition_broadcast(xb[:, xi * XPC:(xi + 1) * XPC], xf[:])

            memt = sp.tile([128, MC], f32)
            nc.scalar.dma_start(out=memt[:], in_=mem_t[:])
            bt = sp.tile([128, MC], f32)
            nc.scalar.dma_start(out=bt[:], in_=b_t[:])

            # ---- fused phases A+B ------------------------------------------
            # Row chunk mc of x_proc feeds exactly column mc of reg_spk, which
            # is the stationary operand of conn chunk ci=mc.  So per chunk:
            # stream W rows (DVE multiply + ACT accumulate-reduce), run the
            # tiny leaky/spike chain on that column, then immediately issue
            # the conn matmuls for that chunk -- phase B hides entirely inside
            # phase A's DMA stream.
            nchunks = NWP
            act_dummy = scp.tile([128, 1], f32, tag="adum", name="act_dummy")
            xps = sp.tile([128, MC], f32)
            tr = sp.tile([128, MC], f32)
            nc.vector.tensor_single_scalar(tr[:], memt[:], 1.0, gt)
            bias_t = sp.tile([128, MC], f32)
            nc.vector.tensor_scalar_mul(bias_t[:], memt[:], 0.5)
            nc.vector.tensor_add(bias_t[:], bias_t[:], bt[:])
            nc.vector.tensor_sub(bias_t[:], bias_t[:], tr[:])
            rm = sp.tile([128, MC], f32)
            rsg = sp.tile([128, MC], f32)
            rs = sp.tile([128, MC], f32)
            rsb = sp.tile([128, MC], bf16)
            red_dummy = sp.tile([128, NWP], f32)
            cneg1 = sp.tile([128, 1], f32)
            nc.vector.memset(cneg1[:], -1.0)
            chalf = sp.tile([128, 1], f32)
            nc.vector.memset(chalf[:], 0.5)
            nin_ps = [pp.tile([1, 512], f32, tag="acc", name=f"ninps{fr}")
                      for fr in range(NFR)]
            def emit_stream(mc):
                # conn tiles for this chunk (hi+lo), issued up front so the
                # DMA stream interleaves with W and stays saturated
                c_pair = []
                for h, src in (("hi", ct_hi), ("lo", ct_lo)):
                    c_tile = cp.tile([128, P], bf16, tag="c",
                                     name=f"c{h}{mc}")
                    nc.sync.dma_start(out=c_tile[:], in_=src[mc, :, :])
                    c_pair.append(c_tile)
                accs4 = sp.tile([128, nchunks], f32, name=f"accs4_{mc}")
                for wpc in range(NWP):
                    w_tile = wp.tile([128, WPIECE], f32, tag="w",
                                     name=f"w{mc}_{wpc}")
                    nc.sync.dma_start(
                        out=w_tile[:],
                        in_=wt[mc, :, wpc * WPIECE:(wpc + 1) * WPIECE])
                    prod = scp.tile([128, WPIECE], f32, tag="prod",
                                    name=f"prod{mc}_{wpc}", bufs=3)
                    for q in range(WPIECE // FW):
                        f0 = q * FW
                        nc.vector.tensor_mul(
                            prod[:, f0:f0 + FW], w_tile[:, f0:f0 + FW],
                            xb[:, wpc * WPIECE + f0: wpc * WPIECE + f0 + FW])
                    # the Copy-activation's only job is the free-dim
                    # accumulate into accs4; out is a stride-0 dummy
                    nc.scalar.activation(
                        act_dummy[:].broadcast_to((128, WPIECE)), prod[:],
                        mybir.ActivationFunctionType.Copy,
                        accum_out=accs4[:, wpc:wpc + 1])
                return c_pair, accs4

            def emit_chain(mc, c_pair, accs4):
                # x_proc column -> leaky -> spike, entirely on the Scalar
                # engine so the Vector engine stays a pure multiply stream:
                #   xps = sum(accs4); rm = xps + bias;
                #   rs = 0.5*sign(rm - 1) + 0.5  (exact 0/1 in fp32)
                col = slice(mc, mc + 1)
                Act = mybir.ActivationFunctionType
                nc.scalar.activation(red_dummy[:], accs4[:], Act.Copy,
                                     accum_out=xps[:, col])
                nc.scalar.activation(rm[:, col], xps[:, col], Act.Identity,
                                     bias=bias_t[:, col], scale=1.0)
                nc.scalar.activation(rsg[:, col], rm[:, col], Act.Sign,
                                     bias=cneg1[:], scale=1.0)
                nc.scalar.activation(rs[:, col], rsg[:, col], Act.Identity,
                                     bias=chalf[:], scale=0.5)
                nc.scalar.activation(rsb[:, col], rs[:, col], Act.Copy)
                for h in range(2):
                    for fr in range(NFR):
                        nc.tensor.matmul(
                            nin_ps[fr][:],
                            rsb[:, col],
                            c_pair[h][:, fr * 512:(fr + 1) * 512],
                            start=(mc == 0 and h == 0),
                            stop=(mc == MC - 1 and h == 1),
                        )

            for mc in range(MC):
                c_pair, accs4 = emit_stream(mc)
                emit_chain(mc, c_pair, accs4)

            nc.scalar.dma_start(out=o_rm[:], in_=rm[:])
            nc.scalar.dma_start(out=o_spk[:], in_=rs[:])
            nint = sp.tile([1, P], f32)
            for fr in range(NFR):
                nc.vector.tensor_copy(nint[:, fr * 512:(fr + 1) * 512],
                                      nin_ps[fr][:])
            nc.scalar.dma_start(out=o_nin[:], in_=nint[:])

    nc.compile()
    return nc


def _get_nc():
    if "nc" not in _COMPILED:
        _COMPILED["nc"] = _build_nc()
    return _COMPILED["nc"]


def _part_major(v):
    """[n*128] -> [128, n] with t[p, c] = v[c*128 + p]."""
    return np.ascontiguousarray(v.reshape(-1, 128).T)


def _prep_inputs(x, mem, W_in, b_in, conn):
    import ml_dtypes
    bf16 = ml_dtypes.bfloat16
    x_f = x.reshape(1, N)
    # conn column-shard, transposed, split to bf16 hi+lo:
    # ct[i][ci, p, m] = conn[m, i*RPC + ci*128 + p]
    C = conn.reshape(P, N_CORES, CC, 128)             # [m, i, ci, p]
    CH = np.ascontiguousarray(C.transpose(1, 2, 3, 0))  # [i, ci, p, m] fp32
    CH_hi = CH.astype(bf16)
    CH_lo = (CH - CH_hi.astype(np.float32)).astype(bf16)
    in_maps = []
    for i in range(N_CORES):
        rows = slice(i * RPC, (i + 1) * RPC)
        in_maps.append({
            "x_f": x_f,
            "mem_t": _part_major(mem[rows]),
            "b_t": _part_major(b_in[rows]),
            "wt": W_in[rows].reshape(MC, 128, N),     # natural layout, no copy
            "ct_hi": CH_hi[i],
            "ct_lo": CH_lo[i],
        })
    return in_maps


def _run_device(in_maps, trace=False, trace_cores=None):
    from concourse.bass_utils import run_bass_kernel_spmd
    nc = _get_nc()
    return run_bass_kernel_spmd(
        nc, in_maps, core_ids=list(range(N_CORES)),
        trace=trace, trace_cores=trace_cores,
    )


def _host_tail(nin, poly_mem, W1, b1, W2, b2):
    h = np.maximum(nin[:, None] * W1 + b1, 0.0).astype(np.float32)
    logits = np.einsum("pk,pmk->pm", h, W2).astype(np.float32) + b2
    m = logits.max(axis=-1, keepdims=True)
    e = np.exp(logits - m)
    probs = (e / e.sum(axis=-1, keepdims=True)).astype(np.float32)
    reset_p = (poly_mem > THRS).astype(np.float32)
    poly_mem_new = (BETAS * poly_mem + nin[:, None] - reset_p * THRS).astype(np.float32)
    poly_spk = (poly_mem_new - THRS > 0).astype(np.float32)
    final = (poly_spk * probs).sum(axis=-1).astype(np.float32)
    return probs, poly_mem_new, final


def kernel(x, mem, poly_mem, W_in, b_in, conn, W1, b1, W2, b2, g_adapt,
           _trace=False, _trace_cores=None):
    x = np.asarray(x, np.float32)
    mem = np.asarray(mem, np.float32)
    poly_mem = np.asarray(poly_mem, np.float32)
    W_in = np.asarray(W_in, np.float32)
    b_in = np.asarray(b_in, np.float32)
    conn = np.asarray(conn, np.float32)
    W1 = np.asarray(W1, np.float32)
    b1 = np.asarray(b1, np.float32)
    W2 = np.asarray(W2, np.float32)
    b2 = np.asarray(b2, np.float32)
    g_adapt = np.asarray(g_adapt, np.float32)

    in_maps = _prep_inputs(x, mem, W_in, b_in, conn)
    res = _run_device(in_maps, trace=_trace, trace_cores=_trace_cores)
    kernel._last_results = res

    reg_mem = np.concatenate(
        [res.results[i]["o_rm"].T.ravel() for i in range(N_CORES)])
    reg_spk = np.concatenate(
        [res.results[i]["o_spk"].T.ravel() for i in range(N_CORES)])
    nin = np.zeros(P, np.float32)
    for i in range(N_CORES):
        nin += res.results[i]["o_nin"].ravel()

    probs, poly_mem_new, final = _host_tail(nin, poly_mem, W1, b1, W2, b2)
    combined = (np.concatenate([reg_spk, final]) * g_adapt[0]).astype(np.float32)
    return combined, reg_mem.astype(np.float32), poly_mem_new, probs


# revision 18
# speedup vs baseline: 1.1818x; 1.1818x over previous
"""Trainium2 Bass kernel for the PolymorphicSNN problem.

Contract: kernel(**inputs) takes FULL (unsharded) numpy inputs and returns the
full output tuple (combined, reg_mem, poly_mem_new, mode_probs), matching the
single-device jax reference.

Sharding (8 NeuronCores, SPMD, no collectives):
  - W_in is row-sharded: core i computes x_proc[i*1024:(i+1)*1024] = W_in[rows] @ x.
    The matvec runs on the Vector engine via the fused tensor_tensor_reduce
    (multiply + per-partition reduce in one pass, fp32-exact) with W in its
    NATURAL layout (rows on partitions) and x broadcast across partitions --
    this streams W at DMA line rate without paying the tensor engine's slow
    fp32 weight loads.
  - The regular-Leaky elementwise + spike threshold runs on-device per shard.
  - conn is column-sharded with the SAME index range: core i computes the
    partial neuron_in = conn[:, rows] @ reg_spk[rows] on the tensor engine
    with reg_spk (exactly 0/1, bf16-exact) as the 1-column stationary operand
    and conn^T split into bf16 hi+lo moving operands (exact to 2^-18).
    The 8 partial vectors (8KB each) are summed on the host during
    unsharding -- this replaces the all-reduce.
  - The tiny per-neuron MLP/softmax/poly-Leaky tail (~100K flops vs 80 MFLOP
    for the matvecs) runs on the host in fp32.
"""

import numpy as np

N = 8192
P = 2048
N_CORES = 8
RPC = N // N_CORES        # rows per core (1024)
MC = RPC // 128           # row chunks per core (8)
FW = 2048                 # free-dim span per DVE reduce op
WPIECE = 4096             # W columns per DMA (2MB pieces)
NWP = N // WPIECE         # W DMA pieces per row chunk (2)
CC = RPC // 128           # conn contraction chunks per core (8)
NFR = P // 512            # conn psum column ranges (4)

SLOPE = 25.0
BETAS = np.array([0.3, 0.7, 0.5], dtype=np.float32)
THRS = np.array([1.0, 0.8, 1.2], dtype=np.float32)

_COMPILED = {}


def _build_nc():
    import concourse.mybir as mybir
    import concourse.tile as tile
    from concourse import bacc

    f32 = mybir.dt.float32
    bf16 = mybir.dt.bfloat16
    gt = mybir.AluOpType.is_gt
    mult = mybir.AluOpType.mult
    add = mybir.AluOpType.add

    nc = bacc.Bacc("TRN2", target_bir_lowering=False, debug=False,
                   num_devices=N_CORES)

    x_f = nc.declare_dram_parameter("x_f", [1, N], f32, isOutput=False)
    mem_t = nc.declare_dram_parameter("mem_t", [128, MC], f32, isOutput=False)
    b_t = nc.declare_dram_parameter("b_t", [128, MC], f32, isOutput=False)
    wt = nc.declare_dram_parameter("wt", [MC, 128, N], f32, isOutput=False)
    ct_hi = nc.declare_dram_parameter("ct_hi", [CC, 128, P], bf16, isOutput=False)
    ct_lo = nc.declare_dram_parameter("ct_lo", [CC, 128, P], bf16, isOutput=False)
    o_rm = nc.declare_dram_parameter("o_rm", [128, MC], f32, isOutput=True)
    o_spk = nc.declare_dram_parameter("o_spk", [128, MC], f32, isOutput=True)
    o_nin = nc.declare_dram_parameter("o_nin", [1, P], f32, isOutput=True)

    with tile.TileContext(nc) as tc:
        with tc.tile_pool(name="small", bufs=1) as sp, \
             tc.tile_pool(name="w", bufs=4) as wp, \
             tc.tile_pool(name="c", bufs=CC) as cp, \
             tc.tile_pool(name="scratch", bufs=2) as scp, \
             tc.tile_pool(name="ps", bufs=4, space="PSUM") as pp:

            # x broadcast across all 128 partitions. Staging DMAs go on the
            # Scalar engine's HWDGE queue so the Sync queue (which carries the
            # bulk W/conn stream) is never blocked behind the broadcasts.
            xb = sp.tile([128, N], f32)
            XPC = N // 4
            for xi in range(4):
                xf = scp.tile([1, XPC], f32, tag="xf", name=f"xf{xi}", bufs=2)
                nc.scalar.dma_start(out=xf[:], in_=x_f[:, xi * XPC:(xi + 1) * XPC])
                nc.gpsimd.partition_broadcast(xb[:, xi * XPC:(xi + 1) * XPC], xf[:])

            memt = sp.tile([128, MC], f32)
            nc.scalar.dma_start(out=memt[:], in_=mem_t[:])
            bt = sp.tile([128, MC], f32)
            nc.scalar.dma_start(out=bt[:], in_=b_t[:])

            # ---- fused phases A+B ------------------------------------------
            # Row chunk mc of x_proc feeds exactly column mc of reg_spk, which
            # is the stationary operand of conn chunk ci=mc.  So per chunk:
            # stream W rows (DVE multiply + ACT accumulate-reduce), run the
            # tiny leaky/spike chain on that column, then immediately issue
            # the conn matmuls for that chunk -- phase B hides entirely inside
            # phase A's DMA stream.
            nchunks = NWP * (WPIECE // FW)
            act_dummy = scp.tile([128, FW], f32, tag="adum", name="act_dummy")
            xps = sp.tile([128, MC], f32)
            tr = sp.tile([128, MC], f32)
            nc.vector.tensor_single_scalar(tr[:], memt[:], 1.0, gt)
            bias_t = sp.tile([128, MC], f32)
            nc.vector.tensor_scalar_mul(bias_t[:], memt[:], 0.5)
            nc.vector.tensor_add(bias_t[:], bias_t[:], bt[:])
            nc.vector.tensor_sub(bias_t[:], bias_t[:], tr[:])
            rm = sp.tile([128, MC], f32)
            rsg = sp.tile([128, MC], f32)
            rs = sp.tile([128, MC], f32)
            rsb = sp.tile([128, MC], bf16)
            red_dummy = sp.tile([128, NWP * (WPIECE // FW)], f32)
            cneg1 = sp.tile([128, 1], f32)
            nc.vector.memset(cneg1[:], -1.0)
            chalf = sp.tile([128, 1], f32)
            nc.vector.memset(chalf[:], 0.5)
            nin_ps = [pp.tile([1, 512], f32, tag="acc", name=f"ninps{fr}")
                      for fr in range(NFR)]
            def emit_stream(mc):
                # conn tiles for this chunk (hi+lo), issued up front so the
                # DMA stream interleaves with W and stays saturated
                c_pair = []
                for h, src in (("hi", ct_hi), ("lo", ct_lo)):
                    c_tile = cp.tile([128, P], bf16, tag="c",
                                     name=f"c{h}{mc}")
                    nc.sync.dma_start(out=c_tile[:], in_=src[mc, :, :])
                    c_pair.append(c_tile)
                accs4 = sp.tile([128, nchunks], f32, name=f"accs4_{mc}")
                for wpc in range(NWP):
                    w_tile = wp.tile([128, WPIECE], f32, tag="w",
                                     name=f"w{mc}_{wpc}")
                    nc.sync.dma_start(
                        out=w_tile[:],
                        in_=wt[mc, :, wpc * WPIECE:(wpc + 1) * WPIECE])
                    for q in range(WPIECE // FW):
                        f0 = q * FW
                        k = wpc * (WPIECE // FW) + q
                        prod = scp.tile([128, FW], f32, tag="prod",
                                        name=f"prod{mc}_{wpc}_{q}", bufs=4)
                        nc.vector.tensor_mul(
                            prod[:], w_tile[:, f0:f0 + FW],
                            xb[:, wpc * WPIECE + f0: wpc * WPIECE + f0 + FW])
                        nc.scalar.activation(
                            act_dummy[:], prod[:],
                            mybir.ActivationFunctionType.Copy,
                            accum_out=accs4[:, k:k + 1])
                return c_pair, accs4

            def emit_chain(mc, c_pair, accs4):
                # x_proc column -> leaky -> spike, entirely on the Scalar
                # engine so the Vector engine stays a pure multiply stream:
                #   xps = sum(accs4); rm = xps + bias;
                #   rs = 0.5*sign(rm - 1) + 0.5  (exact 0/1 in fp32)
                col = slice(mc, mc + 1)
                Act = mybir.ActivationFunctionType
                nc.scalar.activation(red_dummy[:], accs4[:], Act.Copy,
                                     accum_out=xps[:, col])
                nc.scalar.activation(rm[:, col], xps[:, col], Act.Identity,
                                     bias=bias_t[:, col], scale=1.0)
                nc.scalar.activation(rsg[:, col], rm[:, col], Act.Sign,
                                     bias=cneg1[:], scale=1.0)
                nc.scalar.activation(rs[:, col], rsg[:, col], Act.Identity,
                                     bias=chalf[:], scale=0.5)
                nc.scalar.activation(rsb[:, col], rs[:, col], Act.Copy)
                for h in range(2):
                    for fr in range(NFR):
                        nc.tensor.matmul(
                            nin_ps[fr][:],
                            rsb[:, col],
                            c_pair[h][:, fr * 512:(fr + 1) * 512],
                            start=(mc == 0 and h == 0),
                            stop=(mc == MC - 1 and h == 1),
                        )

            for mc in range(MC):
                c_pair, accs4 = emit_stream(mc)
                emit_chain(mc, c_pair, accs4)

            nc.scalar.dma_start(out=o_rm[:], in_=rm[:])
            nc.scalar.dma_start(out=o_spk[:], in_=rs[:])
            nint = sp.tile([1, P], f32)
            for fr in range(NFR):
                nc.vector.tensor_copy(nint[:, fr * 512:(fr + 1) * 512],
                                      nin_ps[fr][:])
            nc.scalar.dma_start(out=o_nin[:], in_=nint[:])

    nc.compile()
    return nc


def _get_nc():
    if "nc" not in _COMPILED:
        _COMPILED["nc"] = _build_nc()
    return _COMPILED["nc"]


def _part_major(v):
    """[n*128] -> [128, n] with t[p, c] = v[c*128 + p]."""
    return np.ascontiguousarray(v.reshape(-1, 128).T)


def _prep_inputs(x, mem, W_in, b_in, conn):
    import ml_dtypes
    bf16 = ml_dtypes.bfloat16
    x_f = x.reshape(1, N)
    # conn column-shard, transposed, split to bf16 hi+lo:
    # ct[i][ci, p, m] = conn[m, i*RPC + ci*128 + p]
    C = conn.reshape(P, N_CORES, CC, 128)             # [m, i, ci, p]
    CH = np.ascontiguousarray(C.transpose(1, 2, 3, 0))  # [i, ci, p, m] fp32
    CH_hi = CH.astype(bf16)
    CH_lo = (CH - CH_hi.astype(np.float32)).astype(bf16)
    in_maps = []
    for i in range(N_CORES):
        rows = slice(i * RPC, (i + 1) * RPC)
        in_maps.append({
            "x_f": x_f,
            "mem_t": _part_major(mem[rows]),
            "b_t": _part_major(b_in[rows]),
            "wt": W_in[rows].reshape(MC, 128, N),     # natural layout, no copy
            "ct_hi": CH_hi[i],
            "ct_lo": CH_lo[i],
        })
    return in_maps


def _run_device(in_maps, trace=False, trace_cores=None):
    from concourse.bass_utils import run_bass_kernel_spmd
    nc = _get_nc()
    return run_bass_kernel_spmd(
        nc, in_maps, core_ids=list(range(N_CORES)),
        trace=trace, trace_cores=trace_cores,
    )


def _host_tail(nin, poly_mem, W1, b1, W2, b2):
    h = np.maximum(nin[:, None] * W1 + b1, 0.0).astype(np.float32)
    logits = np.einsum("pk,pmk->pm", h, W2).astype(np.float32) + b2
    m = logits.max(axis=-1, keepdims=True)
    e = np.exp(logits - m)
    probs = (e / e.sum(axis=-1, keepdims=True)).astype(np.float32)
    reset_p = (poly_mem > THRS).astype(np.float32)
    poly_mem_new = (BETAS * poly_mem + nin[:, None] - reset_p * THRS).astype(np.float32)
    poly_spk = (poly_mem_new - THRS > 0).astype(np.float32)
    final = (poly_spk * probs).sum(axis=-1).astype(np.float32)
    return probs, poly_mem_new, final


def kernel(x, mem, poly_mem, W_in, b_in, conn, W1, b1, W2, b2, g_adapt,
           _trace=False, _trace_cores=None):
    x = np.asarray(x, np.float32)
    mem = np.asarray(mem, np.float32)
    poly_mem = np.asarray(poly_mem, np.float32)
    W_in = np.asarray(W_in, np.float32)
    b_in = np.asarray(b_in, np.float32)
    conn = np.asarray(conn, np.float32)
    W1 = np.asarray(W1, np.float32)
    b1 = np.asarray(b1, np.float32)
    W2 = np.asarray(W2, np.float32)
    b2 = np.asarray(b2, np.float32)
    g_adapt = np.asarray(g_adapt, np.float32)

    in_maps = _prep_inputs(x, mem, W_in, b_in, conn)
    res = _run_device(in_maps, trace=_trace, trace_cores=_trace_cores)
    kernel._last_results = res

    reg_mem = np.concatenate(
        [res.results[i]["o_rm"].T.ravel() for i in range(N_CORES)])
    reg_spk = np.concatenate(
        [res.results[i]["o_spk"].T.ravel() for i in range(N_CORES)])
    nin = np.zeros(P, np.float32)
    for i in range(N_CORES):
        nin += res.results[i]["o_nin"].ravel()

    probs, poly_mem_new, final = _host_tail(nin, poly_mem, W1, b1, W2, b2)
    combined = (np.concatenate([reg_spk, final]) * g_adapt[0]).astype(np.float32)
    return combined, reg_mem.astype(np.float32), poly_mem_new, probs
